# revision 46
# baseline (speedup 1.0000x reference)
"""Trainium2 Bass kernel for a pre-norm transformer block (B=8, N=1024, C=768).

Data-parallel over batch: each of 8 NeuronCores runs the full block for one
batch element.  Activations are feature-major ([feat, tok]) so every matmul
contracts over the partition dim with no on-device transposes.

v2 restructure (vs the straight-line v1):
  - LayerNorm gains are folded into the consuming weights on the host
    (W' = W@diag(ln_w), b' = b + W@ln_b), so LN emits only z=(x-mu)*rs.
  - The block is software-pipelined by token halves: attention for query
    half 1 (Activation-engine-bound softmax exp) runs with proj / LN2 / fc1
    of half 0 interleaved as PE filler work, so the PE never waits on exp.
  - fc1-half-0's gelu is deferred (DVE does the PSUM->SBUF bias add) so the
    Activation engine stays on the Exp table inside the overlap window
    (Exp and Gelu live in different act-function tables; a switch is 1.3us).
  - LN stat matmuls are fp32r (1 row/cycle at free=512); activations and
    weights are bf16 (same PE rate as fp32r, half the DMA and SBUF).
"""

import os
import sys

import numpy as np

for _p in ("/opt/trn_rl_repo", "/root/.axon_site/_ro/trn_rl_repo"):
    if os.path.isdir(_p) and _p not in sys.path:
        sys.path.append(_p)

import concourse.bass as bass  # noqa: E402
import concourse.tile as tile  # noqa: E402
from concourse import bacc, mybir  # noqa: E402
from concourse.bass_utils import run_bass_kernel_spmd  # noqa: E402

F32 = mybir.dt.float32
F32R = mybir.dt.float32r
BF16 = mybir.dt.bfloat16
FP8 = mybir.dt.float8e4

# fp8 e4m3 matmul groups (DoubleRow mode, 2x PE throughput). Weights are
# pre-scaled by WSC on the host to stay out of the fp8 subnormal range;
# the inverse scale is folded into the PSUM->SBUF bias ops.
ATTN_FP8 = os.environ.get("KERNEL_ATTN_FP8", "1") == "1"
FC1_FP8 = os.environ.get("KERNEL_FC1_FP8", "0") == "1"
FC2_FP8 = os.environ.get("KERNEL_FC2_FP8", "1") == "1"
WSC = 32.0

P = 128
D = 768
KD = D // P          # 6 subtiles over the 768 contraction dim
NTOK = 1024
F = 512              # token-half width (matmul free dim)
NHALF = NTOK // F    # 2
H = 12
DK = 64
DFF = 3072
MF1 = DFF // P       # 24
TC = NTOK // P       # 8 token chunks
EPS = 1e-5
N_CORES = 8

ACT = BF16           # activation dtype for matmul operands
WDT = BF16           # weight dtype


def build_program(reps=1):
    nc = bacc.Bacc(
        "TRN2", target_bir_lowering=False, debug=False, num_devices=N_CORES
    )

    din = lambda name, shape, dt=F32: nc.dram_tensor(
        name, shape, dt, kind="ExternalInput"
    ).ap()
    # f32r: full fp32 storage; LN-stat matmuls run at 1 row/cycle.
    xt = din("xt", [P, KD, NTOK], F32R)
    onesr = din("onesr", [P, 1], F32R)
    wqkv = din("wqkv", [18, P, KD, P], FP8 if ATTN_FP8 else WDT)
    bqkv = din("bqkv", [P, 18])
    wproj = din("wproj", [KD, P, KD, P], FP8 if ATTN_FP8 else WDT)
    bproj = din("bproj", [P, KD])
    wfc1 = din("wfc1", [MF1, P, KD, P], FP8 if FC1_FP8 else WDT)
    bfc1 = din("bfc1", [P, MF1])
    wfc2 = din("wfc2", [KD, P, MF1, P], FP8 if FC2_FP8 else WDT)
    bfc2 = din("bfc2", [P, KD])
    yt = nc.dram_tensor("yt", [P, KD, NTOK], F32, kind="ExternalOutput").ap()

    EXP = mybir.ActivationFunctionType.Exp
    GELU = mybir.ActivationFunctionType.Gelu
    SQRT = mybir.ActivationFunctionType.Sqrt
    SUB = mybir.AluOpType.subtract
    ADD = mybir.AluOpType.add
    MUL = mybir.AluOpType.mult
    DR = mybir.MatmulPerfMode.DoubleRow

    attn_dt = FP8 if ATTN_FP8 else ACT      # hT, v_aug, pt, attnT
    h2_dt = FP8 if FC1_FP8 else ACT         # fc1 rhs
    g_dt = FP8 if FC2_FP8 else ACT          # fc2 rhs
    wq_dt = FP8 if ATTN_FP8 else WDT        # wqkv, wproj
    w1_dt = FP8 if FC1_FP8 else WDT
    w2_dt = FP8 if FC2_FP8 else WDT

    with tile.TileContext(nc) as tc:
        # ---- session pools ----
        psum = tc.alloc_tile_pool(name="psum", bufs=8, space="PSUM")
        n_ps = 4 if ATTN_FP8 else 8
        const = tc.alloc_tile_pool(name="const", bufs=1)
        stat = tc.alloc_tile_pool(name="stat", bufs=3)
        bcast = tc.alloc_tile_pool(name="bcast", bufs=4)
        sqp = tc.alloc_tile_pool(name="sqp", bufs=2)
        tmpp = tc.alloc_tile_pool(name="tmpp", bufs=2)
        outp = tc.alloc_tile_pool(name="outp", bufs=2)
        ptp = tc.alloc_tile_pool(name="ptp", bufs=4)
        wstream = tc.alloc_tile_pool(name="wstream", bufs=6)
        w2stream = tc.alloc_tile_pool(name="w2s", bufs=2, side="right")

        def ps_tile():
            return psum.tile([P, F], F32, tag="ps", bufs=n_ps, name="ps")

        def sp_tile():
            # scores for a (head, kc-pair): two PSUM banks, contiguous, so a
            # single exp instruction can read both halves.
            return psum.tile([P, 2, F], F32, tag="sp", bufs=2, name="sp")

        def chain(ps_ap, lhs_fn, rhs_fn, n, fp8):
            """Accumulation chain over n k-subtiles; fp8 uses DoubleRow pairs.

            lhs_fn/rhs_fn(kk, w) -> AP for k-subtiles [kk, kk+w).
            """
            if fp8:
                for kk in range(0, n, 2):
                    nc.tensor.matmul(
                        ps_ap, lhs_fn(kk, 2), rhs_fn(kk, 2),
                        start=(kk == 0), stop=(kk + 2 >= n), perf_mode=DR,
                    )
            else:
                for kk in range(n):
                    nc.tensor.matmul(
                        ps_ap, lhs_fn(kk, 1), rhs_fn(kk, 1),
                        start=(kk == 0), stop=(kk == n - 1),
                    )

        # ---- constants ----
        ones_sb = const.tile([P, 1], F32)
        nc.vector.memset(ones_sb, 1.0)
        ones0_sb = const.tile([P, 4], F32)
        nc.vector.memset(ones0_sb[:, 0:1], 1.0)
        nc.vector.memset(ones0_sb[:, 1:4], 0.0)
        eps_sb = const.tile([P, 1], F32)
        nc.vector.memset(eps_sb, EPS)
        onesr_sb = const.tile([P, 1], F32R, name="onesr_sb")
        nc.sync.dma_start(out=onesr_sb[:], in_=onesr[:])

        def load_const(ap_dram, shape):
            t = const.tile(shape, ap_dram.dtype, name=ap_dram.name + "_sb")
            nc.sync.dma_start(out=t[:], in_=ap_dram[:])
            return t

        bqkv_sb = load_const(bqkv, [P, 18])
        bproj_sb = load_const(bproj, [P, KD])
        bfc1_sb = load_const(bfc1, [P, MF1])
        bfc2_sb = load_const(bfc2, [P, KD])

        # ---- layernorm helpers (z = (x - mu) * rs only; gains folded) ----
        def ln_stats(src_sb, half, sq_act=False):
            # squares on gpsimd: both Act (exp/gelu) and DVE (stores) are
            # busier engines wherever LN runs.
            cols = slice(half * F, (half + 1) * F)
            sum_ps = ps_tile()
            for kk in range(KD):
                nc.tensor.matmul(
                    sum_ps[0:1, :], onesr_sb[:], src_sb[:, kk, cols],
                    start=(kk == 0), stop=(kk == KD - 1),
                )
            sq_ps = ps_tile()
            for kk in range(KD):
                sq = sqp.tile([P, F], F32R, tag="sq", name="sq")
                nc.gpsimd.tensor_tensor(
                    sq[:], src_sb[:, kk, cols], src_sb[:, kk, cols], MUL
                )
                nc.tensor.matmul(
                    sq_ps[0:1, :], onesr_sb[:], sq[:],
                    start=(kk == 0), stop=(kk == KD - 1),
                )
            mu = stat.tile([1, F], F32, tag="st", name="mu")
            nc.vector.tensor_scalar_mul(mu[:], sum_ps[0:1, :], 1.0 / D)
            mu_b = bcast.tile([P, F], F32, tag="bc", name="mu_b")
            nc.gpsimd.partition_broadcast(mu_b[:], mu[:])
            e2 = stat.tile([1, F], F32, tag="st", name="e2")
            nc.vector.tensor_scalar_mul(e2[:], sq_ps[0:1, :], 1.0 / D)
            var = stat.tile([1, F], F32, tag="st", name="var")
            nc.vector.tensor_mul(var[:], mu[:], mu[:])
            nc.vector.tensor_tensor(var[:], e2[:], var[:], SUB)
            sd = stat.tile([1, F], F32, tag="st", name="sd")
            nc.scalar.activation(sd[:], var[:], SQRT, bias=eps_sb[0:1], scale=1.0)
            rs = stat.tile([1, F], F32, tag="st", name="rs")
            nc.vector.reciprocal(rs[:], sd[:])
            rs_b = bcast.tile([P, F], F32, tag="bc", name="rs_b")
            nc.gpsimd.partition_broadcast(rs_b[:], rs[:])
            return mu_b, rs_b

        def ln_norm(src_sb, dst_sb, half, mu_b, rs_b, dst_local=False):
            cols = slice(half * F, (half + 1) * F)
            for kk in range(KD):
                d = dst_sb[:, kk, :] if dst_local else dst_sb[:, kk, cols]
                # subtract on gpsimd (otherwise idle), multiply on DVE; the
                # bf16 staging tile also gives fp8 dst a single quantization.
                t = tmpp.tile([P, F], BF16, tag="t", name="t")
                nc.gpsimd.tensor_tensor(
                    t, src_sb[:, kk, cols], mu_b[:], SUB
                )
                nc.vector.tensor_mul(d, t, rs_b[:])

        for _rep in range(reps):
            # ---- per-rep pools (left) ----
            xt_pool = tc.alloc_tile_pool(name="xt", bufs=1)
            attn_pool = tc.alloc_tile_pool(name="attn", bufs=1)
            qk_pool = tc.alloc_tile_pool(name="qk", bufs=1)
            vaug_pool = tc.alloc_tile_pool(name="vaug", bufs=1)
            h_pool = tc.alloc_tile_pool(name="h", bufs=1)
            wv_pool = tc.alloc_tile_pool(name="wv", bufs=1)
            # ---- per-rep pools (right) ----
            x2_pool = tc.alloc_tile_pool(name="x2", bufs=1, side="right")
            h2_pool = tc.alloc_tile_pool(name="h2", bufs=1, side="right")
            g_pool = tc.alloc_tile_pool(
                name="g", bufs=(2 if FC2_FP8 else 1), side="right"
            )

            xt_sb = xt_pool.tile([P, KD, NTOK], F32R, name="xt_sb")
            attnT = attn_pool.tile([P, KD, NTOK], attn_dt, name="attnT")
            # k chunks span all tokens; q chunks are per-half tiles (q-h1 all
            # six live through the overlap window, q-h0 transient).
            kT = qk_pool.tile([P, KD, NTOK], ACT, name="kT")
            # DK+4 columns: col 64 = ones (softmax denominator rides the PV
            # matmul), cols 65-67 zero pad (dual-fp8 Ldweights wants 32-bit
            # aligned row strides).
            v_aug = vaug_pool.tile([P, TC, H, DK + 4], attn_dt, name="v_aug")
            hT = h_pool.tile([P, KD, NTOK], attn_dt, name="hT")
            wv_sb = wv_pool.tile([P, KD, KD, P], wq_dt, name="wv_sb")
            x2T = x2_pool.tile([P, KD, NTOK], F32R, name="x2T")
            # h2 / g of the two halves share one slot each: h1's writes are
            # dep-serialized after h0's last reader, which matches the
            # schedule (fc1-h0 / fc2-h0 finish before the h1 tail runs).
            h2 = [
                h2_pool.tile([P, KD, F], h2_dt, tag="h2", name=f"h2_{i}")
                for i in range(2)
            ]
            gT0 = g_pool.tile([P, MF1, F], g_dt, tag="g", name="gT0")
            gT1 = g_pool.tile([P, MF1, F], g_dt, tag="g", name="gT1")
            # bf16 staging for fc1-h0's deferred (post-window) gelu, so fp8
            # gT0 is quantized once, after the nonlinearity.  bf16 gT0 can
            # take the gelu in place (no second quantization to avoid).
            g_pre = (
                g_pool.tile([P, MF1, F], ACT, tag="gpre", bufs=1, name="g_pre")
                if FC2_FP8 else gT0
            )

            # =========== prologue: load x, LN1, q/k/v ===========
            for half in range(NHALF):
                for kk in range(KD):
                    cols = slice(half * F, (half + 1) * F)
                    nc.sync.dma_start(out=xt_sb[:, kk, cols], in_=xt[:, kk, cols])

            # LN1 h0 stats, then h1 stats (PE), normalizes on DVE overlap qk.
            st0 = ln_stats(xt_sb, 0)
            st1 = ln_stats(xt_sb, 1)
            ln_norm(xt_sb, hT, 0, *st0)

            qk_w = {}
            q_tiles = {}

            def load_qk_w(j):
                wtq = wstream.tile([P, KD, P], wq_dt, tag="w", name="wtq")
                nc.sync.dma_start(out=wtq[:], in_=wqkv[j])
                wtk = wstream.tile([P, KD, P], wq_dt, tag="w", name="wtk")
                nc.sync.dma_start(out=wtk[:], in_=wqkv[KD + j])
                qk_w[j] = (wtq, wtk)

            def qk_store(dst, ps, m):
                # PSUM -> SBUF with bias; undoes the fp8 weight pre-scale.
                if ATTN_FP8:
                    nc.vector.tensor_scalar(
                        dst, ps[:], scalar1=1.0 / WSC,
                        scalar2=bqkv_sb[:, m : m + 1], op0=MUL, op1=ADD,
                    )
                else:
                    nc.vector.tensor_scalar(
                        dst, ps[:],
                        scalar1=bqkv_sb[:, m : m + 1], scalar2=None, op0=ADD,
                    )

            def wsl(t, kk, w):
                return t[:, kk, :] if w == 1 else t[:, kk : kk + w, :]

            def emit_k_chunk(j, half, wt):
                cols = slice(half * F, (half + 1) * F)
                ps = ps_tile()
                chain(ps[:], lambda kk, w: wsl(wt, kk, w),
                      lambda kk, w: hT[:, kk, cols] if w == 1
                      else hT[:, kk : kk + w, cols],
                      KD, ATTN_FP8)
                qk_store(kT[:, j, cols], ps, KD + j)

            def emit_q_chunk(j, half, wt):
                cols = slice(half * F, (half + 1) * F)
                ps = ps_tile()
                chain(ps[:], lambda kk, w: wsl(wt, kk, w),
                      lambda kk, w: hT[:, kk, cols] if w == 1
                      else hT[:, kk : kk + w, cols],
                      KD, ATTN_FP8)
                qt = ptp.tile([P, F], ACT, tag=f"q{half}",
                              bufs=(3 if half == 0 else 6), name="qt")
                q_tiles[(j, half)] = qt
                qk_store(qt[:], ps, j)

            # v projection (token-major); chunk t uses only tokens of its half.
            nc.sync.dma_start(
                out=wv_sb[:], in_=wqkv[12:18].rearrange("m p kk o -> p kk m o")
            )
            nc.vector.tensor_copy(
                out=v_aug[:, :, :, DK : DK + 4],
                in_=ones0_sb[:, None, None, :].to_broadcast([P, TC, H, 4]),
            )

            def emit_v_chunk(t):
                trange = slice(t * P, (t + 1) * P)
                ps2 = (ps_tile(), ps_tile())
                for nn in range(2):  # 512 + 256 of the 768 v features
                    nw = 512 if nn == 0 else 256
                    chain(
                        ps2[nn][:, :nw],
                        lambda kk, w: hT[:, kk, trange] if w == 1
                        else hT[:, kk : kk + w, trange],
                        lambda kk, w, nn=nn, nw=nw:
                        wv_sb[:, kk, 4 * nn : 4 * nn + nw // P, :] if w == 1
                        else wv_sb[:, kk : kk + w, 4 * nn : 4 * nn + nw // P, :],
                        KD, ATTN_FP8,
                    )
                for nn in range(2):
                    nw = 512 if nn == 0 else 256
                    hw = nw // DK
                    # v bias is folded into the proj bias on the host
                    # (proj(O + b) = proj(O) + Wproj@b), so this is a pure
                    # copy (gpsimd cannot read PSUM, so it stays on DVE).
                    nc.vector.tensor_copy(
                        out=v_aug[:, t, nn * 8 : nn * 8 + hw, 0:DK],
                        in_=ps2[nn][:, :nw].rearrange("p (h d) -> p h d", d=DK),
                    )

            load_qk_w(0)
            load_qk_w(1)
            # PE order: k/q h0 (after norm h0), first v chunks, k h1; the
            # remaining v chunks ride as fillers inside head pair 0 so the
            # Act engine starts on exps as early as possible.
            for j in (0, 1):
                emit_k_chunk(j, 0, qk_w[j][1])
                emit_q_chunk(j, 0, qk_w[j][0])
            ln_norm(xt_sb, hT, 1, *st1)
            emit_v_chunk(0)
            emit_v_chunk(1)
            for j in (0, 1):
                emit_k_chunk(j, 1, qk_w[j][1])

            # =========== attention ===========
            def emit_head_pair(j, half, fillers=()):
                fillers = list(fillers)
                cols = slice(half * F, (half + 1) * F)
                pranges = (slice(0, DK), slice(DK, P))
                q_sb = q_tiles.pop((j, half))
                o_ps = (ps_tile(), ps_tile())
                if ATTN_FP8:
                    for kcp in range(TC // 2):
                        sps = [sp_tile(), sp_tile()]
                        for par in (0, 1):
                            kc = 2 * kcp + par
                            for hi in (0, 1):
                                pr = pranges[hi]
                                nc.tensor.matmul(
                                    sps[hi][:, par, :],
                                    kT[pr, j, kc * P : (kc + 1) * P],
                                    q_sb[pr, :],
                                    start=True, stop=True,
                                )
                        pt_pairs = [
                            ptp.tile([P, 2, F], FP8, tag="pt8", name="ptpair")
                            for _ in (0, 1)
                        ]
                        for hi in (0, 1):
                            # one exp over both kc's scores (2 PSUM banks)
                            nc.scalar.activation(
                                pt_pairs[hi][:, :, :], sps[hi][:, :, :], EXP,
                                scale=float(DK) ** -0.5,
                            )
                            nc.tensor.matmul(
                                o_ps[hi][0 : DK + 4, :],
                                v_aug[:, 2 * kcp : 2 * kcp + 2, 2 * j + hi, :],
                                pt_pairs[hi][:, :, :],
                                start=(kcp == 0), stop=(kcp == TC // 2 - 1),
                                perf_mode=DR,
                            )
                        for _ in (0, 1):
                            if fillers:
                                fillers.pop(0)()
                else:
                    for kc in range(TC):
                        pts = []
                        for hi in (0, 1):
                            pr = pranges[hi]
                            s_ps = ps_tile()
                            nc.tensor.matmul(
                                s_ps[:],
                                kT[pr, j, kc * P : (kc + 1) * P],
                                q_sb[pr, :],
                                start=True, stop=True,
                            )
                            pt = ptp.tile([P, F], ACT, tag="pt", name="pt")
                            nc.scalar.activation(
                                pt[:], s_ps[:], EXP, scale=float(DK) ** -0.5
                            )
                            pts.append(pt)
                        for hi in (0, 1):
                            nc.tensor.matmul(
                                o_ps[hi][0 : DK + 4, :],
                                v_aug[:, kc, 2 * j + hi, :],
                                pts[hi][:],
                                start=(kc == 0), stop=(kc == TC - 1),
                            )
                        if fillers:
                            fillers.pop(0)()
                while fillers:
                    fillers.pop(0)()
                for hi in (0, 1):
                    rec = stat.tile([1, F], F32, tag="st", name="rec")
                    nc.vector.reciprocal(rec[:], o_ps[hi][DK : DK + 1, :])
                    rec_b = bcast.tile([DK, F], F32, tag="bc64", bufs=2, name="rec_b")
                    nc.gpsimd.partition_broadcast(rec_b[:], rec[:])
                    nc.vector.tensor_mul(
                        attnT[pranges[hi], j, cols], o_ps[hi][0:DK, :], rec_b[:]
                    )

            wp_tiles = {}

            def load_wp(m):
                wp = wstream.tile([P, KD, P], wq_dt, tag="w", name="wp")
                nc.sync.dma_start(out=wp[:], in_=wproj[m])
                wp_tiles[m] = wp

            # ---- attn h0: head pairs with v chunks (j=0) and q/k chunk
            # production (j>=1) as fillers ----
            for j in range(KD):
                if j == 0:
                    fillers = [lambda t=t: emit_v_chunk(t) for t in range(2, TC)]
                else:
                    fillers = [lambda j=j: emit_q_chunk(j, 1, qk_w[j][0])]
                    if j == 1:
                        fillers.insert(
                            0, lambda: emit_q_chunk(0, 1, qk_w[0][0])
                        )
                    if j + 1 < KD:
                        jj = j + 1

                        def _load_and_k0(jj=jj):
                            load_qk_w(jj)
                            emit_k_chunk(jj, 0, qk_w[jj][1])

                        fillers += [
                            _load_and_k0,
                            lambda jj=jj: emit_k_chunk(jj, 1, qk_w[jj][1]),
                            lambda jj=jj: emit_q_chunk(jj, 0, qk_w[jj][0]),
                        ]
                    else:
                        fillers += [lambda m=m: load_wp(m) for m in range(KD)]
                emit_head_pair(j, 0, fillers)

            # ---- window: attn h1 with proj/LN2/fc1 of h0 interleaved ----
            def emit_proj_chunk(m, half):
                cols = slice(half * F, (half + 1) * F)
                wp = wp_tiles.pop(m)
                ps = ps_tile()
                chain(ps[:], lambda kk, w: wsl(wp, kk, w),
                      lambda kk, w: attnT[:, kk, cols] if w == 1
                      else attnT[:, kk : kk + w, cols],
                      KD, ATTN_FP8)
                if ATTN_FP8:
                    # attnT carries WSC (v scale), wproj carries WSC.
                    nc.vector.tensor_scalar(
                        x2T[:, m, cols], ps[:], scalar1=1.0 / (WSC * WSC),
                        scalar2=bproj_sb[:, m : m + 1], op0=MUL, op1=ADD,
                    )
                else:
                    nc.vector.tensor_scalar(
                        x2T[:, m, cols], ps[:],
                        scalar1=bproj_sb[:, m : m + 1], scalar2=None, op0=ADD,
                    )
                nc.gpsimd.tensor_tensor(
                    x2T[:, m, cols], x2T[:, m, cols], xt_sb[:, m, cols], ADD
                )

            def emit_fc1_chunk(m, half, deferred_gelu):
                wt = wstream.tile([P, KD, P], w1_dt, tag="w", name="wt")
                nc.sync.dma_start(out=wt[:], in_=wfc1[m])
                ps = ps_tile()
                h2h = h2[half]
                chain(ps[:], lambda kk, w: wsl(wt, kk, w),
                      lambda kk, w: wsl(h2h, kk, w),
                      KD, FC1_FP8)
                gT = gT0 if half == 0 else gT1
                if deferred_gelu:
                    # DVE bias add into bf16 staging; gelu applied
                    # post-window (keeps Act on Exp in the overlap window).
                    if FC1_FP8:
                        nc.vector.tensor_scalar(
                            g_pre[:, m, :], ps[:], scalar1=1.0 / WSC,
                            scalar2=bfc1_sb[:, m : m + 1], op0=MUL, op1=ADD,
                        )
                    else:
                        nc.vector.tensor_scalar(
                            g_pre[:, m, :], ps[:],
                            scalar1=bfc1_sb[:, m : m + 1], scalar2=None, op0=ADD,
                        )
                else:
                    nc.scalar.activation(
                        gT[:, m, :], ps[:], GELU,
                        bias=bfc1_sb[:, m : m + 1],
                        scale=(1.0 / WSC if FC1_FP8 else 1.0),
                    )

            ln2_st = {}

            def ln2_stats_h0():
                ln2_st[0] = ln_stats(x2T, 0)

            def ln2_norm_h0():
                ln_norm(x2T, h2[0], 0, *ln2_st[0], dst_local=True)

            for j in range(KD):
                if j == 0:
                    fillers = [lambda m=m: emit_proj_chunk(m, 0) for m in range(KD)]
                elif j == 1:
                    fillers = [ln2_stats_h0, ln2_norm_h0]
                else:
                    fillers = [
                        lambda m=m: emit_fc1_chunk(m, 0, True)
                        for m in range(6 * (j - 2), 6 * (j - 2) + 6)
                    ]
                emit_head_pair(j, 1, fillers)

            # ---- post-window: gelu h0, proj h1, LN2 h1, fc2 h0, MLP h1 ----
            # zdep: a zero bias tile data-dependent on the last attention
            # output, pinning the deferred gelus after the exp window so the
            # scheduler can't interleave them (Exp/Gelu table thrash).
            zdep = stat.tile([P, 1], F32, tag="zdep", bufs=1, name="zdep")
            nc.vector.tensor_scalar_mul(zdep[:], attnT[:, KD - 1, NTOK - 1 : NTOK], 0.0)
            for m in range(KD):
                for mm in range(4 * m, 4 * m + 4):
                    nc.scalar.activation(
                        gT0[:, mm, :], g_pre[:, mm, :], GELU,
                        bias=zdep[:, 0:1], scale=1.0,
                    )
                load_wp(m)
                emit_proj_chunk(m, 1)
            st2 = ln_stats(x2T, 1)
            ln_norm(x2T, h2[1], 1, *st2, dst_local=True)

            def emit_fc2_chunk(m, half):
                cols = slice(half * F, (half + 1) * F)
                gT = gT0 if half == 0 else gT1
                w2 = w2stream.tile([P, MF1, P], w2_dt, tag="w2", name="w2")
                nc.sync.dma_start(out=w2[:], in_=wfc2[m])
                ps = ps_tile()
                chain(ps[:], lambda kk, w: wsl(w2, kk, w),
                      lambda kk, w: wsl(gT, kk, w),
                      MF1, FC2_FP8)
                yo = outp.tile([P, F], F32, tag="yo", name="yo")
                if FC2_FP8:
                    nc.vector.tensor_scalar(
                        yo[:], ps[:], scalar1=1.0 / WSC,
                        scalar2=bfc2_sb[:, m : m + 1], op0=MUL, op1=ADD,
                    )
                else:
                    nc.vector.tensor_scalar(
                        yo[:], ps[:],
                        scalar1=bfc2_sb[:, m : m + 1], scalar2=None, op0=ADD,
                    )
                nc.vector.tensor_add(yo[:], yo[:], x2T[:, m, cols])
                nc.sync.dma_start(out=yt[:, m, cols], in_=yo[:])

            for m in range(KD):
                emit_fc2_chunk(m, 0)
            for m in range(MF1):
                emit_fc1_chunk(m, 1, False)
            for m in range(KD):
                emit_fc2_chunk(m, 1)

            g_pool.release()
            h2_pool.release()
            x2_pool.release()
            wv_pool.release()
            h_pool.release()
            vaug_pool.release()
            qk_pool.release()
            attn_pool.release()
            xt_pool.release()

        w2stream.release()
        wstream.release()
        ptp.release()
        outp.release()
        tmpp.release()
        sqp.release()
        bcast.release()
        stat.release()
        const.release()
        psum.release()

    nc.compile()
    return nc


def _retile_w(w_t, mtiles):
    """[out, in] torch-convention weight -> [mtiles, P, in//P, P] chunk layout.

    chunk[m, p, kk, o] = w_t[m*P + o, kk*P + p]
    """
    out_dim, in_dim = w_t.shape
    a = w_t.reshape(mtiles, P, in_dim // P, P).transpose(0, 3, 2, 1)
    return np.ascontiguousarray(a)


def _vec_tile(v):
    """[n] -> [P, n//P] with t[p, m] = v[m*P + p]."""
    return np.ascontiguousarray(v.reshape(-1, P).T)


_NC_CACHE = {}


def _get_nc():
    if "nc" not in _NC_CACHE:
        _NC_CACHE["nc"] = build_program()
    return _NC_CACHE["nc"]


def prep_inputs(x, ln1_w, ln1_b, qkv_w, qkv_b, proj_w, proj_b,
                ln2_w, ln2_b, fc1_w, fc1_b, fc2_w, fc2_b):
    import ml_dtypes

    wdt_np = np.dtype(ml_dtypes.bfloat16)
    fp8_np = np.dtype(mybir.dt.np(FP8))
    wq_np = fp8_np if ATTN_FP8 else wdt_np
    w1_np = fp8_np if FC1_FP8 else wdt_np
    w2_np = fp8_np if FC2_FP8 else wdt_np
    wq_sc = WSC if ATTN_FP8 else 1.0
    w1_sc = WSC if FC1_FP8 else 1.0
    w2_sc = WSC if FC2_FP8 else 1.0
    f32 = lambda a: np.asarray(a, dtype=np.float32)
    x = f32(x)

    # Fold LN gains/biases into the consuming weights, and the v bias into
    # the proj bias: attn_out(v + b) = attn_out(v) + b, so
    # proj(attn + b) + proj_b = proj(attn) + (proj_b + proj_w @ b).
    qkv_w_f = f32(qkv_w) * f32(ln1_w)[None, :]
    qkv_b_f = f32(qkv_b) + f32(qkv_w) @ f32(ln1_b)
    fc1_w_f = f32(fc1_w) * f32(ln2_w)[None, :]
    fc1_b_f = f32(fc1_b) + f32(fc1_w) @ f32(ln2_b)
    proj_b_f = f32(proj_b) + f32(proj_w) @ qkv_b_f[1536:]

    shared = {
        "onesr": np.ones((P, 1), dtype=np.float32),
        "wqkv": _retile_w(qkv_w_f * wq_sc, 18).astype(wq_np),
        "bqkv": _vec_tile(qkv_b_f),
        "wproj": _retile_w(f32(proj_w) * wq_sc, KD).astype(wq_np),
        "bproj": _vec_tile(proj_b_f),
        "wfc1": _retile_w(fc1_w_f * w1_sc, MF1).astype(w1_np),
        "bfc1": _vec_tile(fc1_b_f),
        "wfc2": _retile_w(f32(fc2_w) * w2_sc, KD).astype(w2_np),
        "bfc2": _vec_tile(f32(fc2_b)),
    }
    in_maps = []
    for b in range(N_CORES):
        m = dict(shared)
        # xt[p, s, n] = x[b, n, s*P + p]
        m["xt"] = np.ascontiguousarray(x[b].reshape(NTOK, KD, P).transpose(2, 1, 0))
        in_maps.append(m)
    return in_maps


def kernel(**inputs):
    nc = _get_nc()
    in_maps = prep_inputs(**inputs)
    res = run_bass_kernel_spmd(nc, in_maps, list(range(N_CORES)))
    outs = []
    for b in range(N_CORES):
        ytile = res.results[b]["yt"]  # [P, KD, NTOK]
        outs.append(ytile.transpose(2, 1, 0).reshape(NTOK, D))
    return np.stack(outs).astype(np.float32)


# revision 49
# speedup vs baseline: 1.1502x; 1.1502x over previous
"""Trainium2 Bass kernel for a pre-norm transformer block (B=8, N=1024, C=768).

Data-parallel over batch: each of 8 NeuronCores runs the full block for one
batch element.  Activations are feature-major ([feat, tok]) so every matmul
contracts over the partition dim with no on-device transposes.

v2 restructure (vs the straight-line v1):
  - LayerNorm gains are folded into the consuming weights on the host
    (W' = W@diag(ln_w), b' = b + W@ln_b), so LN emits only z=(x-mu)*rs.
  - The block is software-pipelined by token halves: attention for query
    half 1 (Activation-engine-bound softmax exp) runs with proj / LN2 / fc1
    of half 0 interleaved as PE filler work, so the PE never waits on exp.
  - fc1-half-0's gelu is deferred (DVE does the PSUM->SBUF bias add) so the
    Activation engine stays on the Exp table inside the overlap window
    (Exp and Gelu live in different act-function tables; a switch is 1.3us).
  - LN stat matmuls are fp32r (1 row/cycle at free=512); scores/fc1 run in
    bf16; qkv/PV/proj and fc2 run fp8 e4m3 in DoubleRow mode (0.5
    cycles/row) with x32 weight pre-scaling to dodge fp8 subnormals.
    fc1+fc2 both in fp8 would breach the 2e-2 error gate, so fc1 stays bf16.
  - The v bias is folded into the proj bias (proj(O+b) = proj(O)+Wproj@b)
    and LN squares/subtracts run on gpsimd, keeping DVE off the critical
    path in the prologue.
"""

import os
import sys

import numpy as np

for _p in ("/opt/trn_rl_repo", "/root/.axon_site/_ro/trn_rl_repo"):
    if os.path.isdir(_p) and _p not in sys.path:
        sys.path.append(_p)

import concourse.bass as bass  # noqa: E402
import concourse.tile as tile  # noqa: E402
from concourse import bacc, mybir  # noqa: E402
from concourse.bass_utils import run_bass_kernel_spmd  # noqa: E402

F32 = mybir.dt.float32
F32R = mybir.dt.float32r
BF16 = mybir.dt.bfloat16
FP8 = mybir.dt.float8e4

# fp8 e4m3 matmul groups (DoubleRow mode, 2x PE throughput). Weights are
# pre-scaled by WSC on the host to stay out of the fp8 subnormal range;
# the inverse scale is folded into the PSUM->SBUF bias ops.
ATTN_FP8 = os.environ.get("KERNEL_ATTN_FP8", "1") == "1"
FC1_FP8 = os.environ.get("KERNEL_FC1_FP8", "0") == "1"
FC2_FP8 = os.environ.get("KERNEL_FC2_FP8", "1") == "1"
WSC = 32.0

P = 128
D = 768
KD = D // P          # 6 subtiles over the 768 contraction dim
NTOK = 1024
F = 512              # token-half width (matmul free dim)
NHALF = NTOK // F    # 2
H = 12
DK = 64
DFF = 3072
MF1 = DFF // P       # 24
TC = NTOK // P       # 8 token chunks
EPS = 1e-5
N_CORES = 8

ACT = BF16           # activation dtype for matmul operands
WDT = BF16           # weight dtype


def build_program(reps=1):
    nc = bacc.Bacc(
        "TRN2", target_bir_lowering=False, debug=False, num_devices=N_CORES
    )

    din = lambda name, shape, dt=F32: nc.dram_tensor(
        name, shape, dt, kind="ExternalInput"
    ).ap()
    # f32r: full fp32 storage; LN-stat matmuls run at 1 row/cycle.
    xt = din("xt", [P, KD, NTOK], F32R)
    onesr = din("onesr", [P, 1], F32R)
    wqkv = din("wqkv", [18, P, KD, P], FP8 if ATTN_FP8 else WDT)
    bqkv = din("bqkv", [P, 18])
    wproj = din("wproj", [KD, P, KD, P], FP8 if ATTN_FP8 else WDT)
    bproj = din("bproj", [P, KD])
    wfc1 = din("wfc1", [MF1, P, KD, P], FP8 if FC1_FP8 else WDT)
    bfc1 = din("bfc1", [P, MF1])
    wfc2 = din("wfc2", [KD, P, MF1, P], FP8 if FC2_FP8 else WDT)
    bfc2 = din("bfc2", [P, KD])
    yt = nc.dram_tensor("yt", [P, KD, NTOK], F32, kind="ExternalOutput").ap()

    EXP = mybir.ActivationFunctionType.Exp
    GELU = mybir.ActivationFunctionType.Gelu
    SQRT = mybir.ActivationFunctionType.Sqrt
    SUB = mybir.AluOpType.subtract
    ADD = mybir.AluOpType.add
    MUL = mybir.AluOpType.mult
    DR = mybir.MatmulPerfMode.DoubleRow

    attn_dt = FP8 if ATTN_FP8 else ACT      # hT, v_aug, pt, attnT
    h2_dt = FP8 if FC1_FP8 else ACT         # fc1 rhs
    g_dt = FP8 if FC2_FP8 else ACT          # fc2 rhs
    wq_dt = FP8 if ATTN_FP8 else WDT        # wqkv, wproj
    w1_dt = FP8 if FC1_FP8 else WDT
    w2_dt = FP8 if FC2_FP8 else WDT

    with tile.TileContext(nc) as tc:
        # ---- session pools ----
        psum = tc.alloc_tile_pool(name="psum", bufs=8, space="PSUM")
        n_ps = 4 if ATTN_FP8 else 8
        const = tc.alloc_tile_pool(name="const", bufs=1)
        stat = tc.alloc_tile_pool(name="stat", bufs=3)
        bcast = tc.alloc_tile_pool(name="bcast", bufs=4)
        sqp = tc.alloc_tile_pool(name="sqp", bufs=2)
        tmpp = tc.alloc_tile_pool(name="tmpp", bufs=2)
        outp = tc.alloc_tile_pool(name="outp", bufs=2)
        ptp = tc.alloc_tile_pool(name="ptp", bufs=4)
        wstream = tc.alloc_tile_pool(name="wstream", bufs=6)
        w2stream = tc.alloc_tile_pool(name="w2s", bufs=2, side="right")

        def ps_tile():
            return psum.tile([P, F], F32, tag="ps", bufs=n_ps, name="ps")

        def sp_tile():
            # scores for a (head, kc-pair): two PSUM banks, contiguous, so a
            # single exp instruction can read both halves.
            return psum.tile([P, 2, F], F32, tag="sp", bufs=2, name="sp")

        def chain(ps_ap, lhs_fn, rhs_fn, n, fp8):
            """Accumulation chain over n k-subtiles; fp8 uses DoubleRow pairs.

            lhs_fn/rhs_fn(kk, w) -> AP for k-subtiles [kk, kk+w).
            """
            if fp8:
                for kk in range(0, n, 2):
                    nc.tensor.matmul(
                        ps_ap, lhs_fn(kk, 2), rhs_fn(kk, 2),
                        start=(kk == 0), stop=(kk + 2 >= n), perf_mode=DR,
                    )
            else:
                for kk in range(n):
                    nc.tensor.matmul(
                        ps_ap, lhs_fn(kk, 1), rhs_fn(kk, 1),
                        start=(kk == 0), stop=(kk == n - 1),
                    )

        # ---- constants ----
        ones_sb = const.tile([P, 1], F32)
        nc.vector.memset(ones_sb, 1.0)
        ones0_sb = const.tile([P, 4], F32)
        nc.vector.memset(ones0_sb[:, 0:1], 1.0)
        nc.vector.memset(ones0_sb[:, 1:4], 0.0)
        eps_sb = const.tile([P, 1], F32)
        nc.vector.memset(eps_sb, EPS)
        onesr_sb = const.tile([P, 1], F32R, name="onesr_sb")
        nc.sync.dma_start(out=onesr_sb[:], in_=onesr[:])

        def load_const(ap_dram, shape):
            t = const.tile(shape, ap_dram.dtype, name=ap_dram.name + "_sb")
            nc.sync.dma_start(out=t[:], in_=ap_dram[:])
            return t

        bqkv_sb = load_const(bqkv, [P, 18])
        bproj_sb = load_const(bproj, [P, KD])
        bfc1_sb = load_const(bfc1, [P, MF1])
        bfc2_sb = load_const(bfc2, [P, KD])

        # ---- layernorm helpers (z = (x - mu) * rs only; gains folded) ----
        def ln_stats(src_sb, half, sq_act=False):
            # squares on gpsimd: both Act (exp/gelu) and DVE (stores) are
            # busier engines wherever LN runs.
            cols = slice(half * F, (half + 1) * F)
            sum_ps = ps_tile()
            for kk in range(KD):
                nc.tensor.matmul(
                    sum_ps[0:1, :], onesr_sb[:], src_sb[:, kk, cols],
                    start=(kk == 0), stop=(kk == KD - 1),
                )
            sq_ps = ps_tile()
            for kk in range(KD):
                sq = sqp.tile([P, F], F32R, tag="sq", name="sq")
                nc.gpsimd.tensor_tensor(
                    sq[:], src_sb[:, kk, cols], src_sb[:, kk, cols], MUL
                )
                nc.tensor.matmul(
                    sq_ps[0:1, :], onesr_sb[:], sq[:],
                    start=(kk == 0), stop=(kk == KD - 1),
                )
            mu = stat.tile([1, F], F32, tag="st", name="mu")
            nc.vector.tensor_scalar_mul(mu[:], sum_ps[0:1, :], 1.0 / D)
            mu_b = bcast.tile([P, F], F32, tag="bc", name="mu_b")
            nc.gpsimd.partition_broadcast(mu_b[:], mu[:])
            e2 = stat.tile([1, F], F32, tag="st", name="e2")
            nc.vector.tensor_scalar_mul(e2[:], sq_ps[0:1, :], 1.0 / D)
            var = stat.tile([1, F], F32, tag="st", name="var")
            nc.vector.tensor_mul(var[:], mu[:], mu[:])
            nc.vector.tensor_tensor(var[:], e2[:], var[:], SUB)
            sd = stat.tile([1, F], F32, tag="st", name="sd")
            nc.scalar.activation(sd[:], var[:], SQRT, bias=eps_sb[0:1], scale=1.0)
            rs = stat.tile([1, F], F32, tag="st", name="rs")
            nc.vector.reciprocal(rs[:], sd[:])
            rs_b = bcast.tile([P, F], F32, tag="bc", name="rs_b")
            nc.gpsimd.partition_broadcast(rs_b[:], rs[:])
            return mu_b, rs_b

        def ln_norm(src_sb, dst_sb, half, mu_b, rs_b, dst_local=False):
            cols = slice(half * F, (half + 1) * F)
            for kk in range(KD):
                d = dst_sb[:, kk, :] if dst_local else dst_sb[:, kk, cols]
                # subtract on gpsimd (otherwise idle), multiply on DVE; the
                # bf16 staging tile also gives fp8 dst a single quantization.
                t = tmpp.tile([P, F], BF16, tag="t", name="t")
                nc.gpsimd.tensor_tensor(
                    t, src_sb[:, kk, cols], mu_b[:], SUB
                )
                nc.vector.tensor_mul(d, t, rs_b[:])

        for _rep in range(reps):
            # ---- per-rep pools (left) ----
            xt_pool = tc.alloc_tile_pool(name="xt", bufs=1)
            attn_pool = tc.alloc_tile_pool(name="attn", bufs=1)
            qk_pool = tc.alloc_tile_pool(name="qk", bufs=1)
            vaug_pool = tc.alloc_tile_pool(name="vaug", bufs=1)
            h_pool = tc.alloc_tile_pool(name="h", bufs=1)
            wv_pool = tc.alloc_tile_pool(name="wv", bufs=1)
            # ---- per-rep pools (right) ----
            x2_pool = tc.alloc_tile_pool(name="x2", bufs=1, side="right")
            h2_pool = tc.alloc_tile_pool(name="h2", bufs=1, side="right")
            g_pool = tc.alloc_tile_pool(
                name="g", bufs=(2 if FC2_FP8 else 1), side="right"
            )

            xt_sb = xt_pool.tile([P, KD, NTOK], F32R, name="xt_sb")
            attnT = attn_pool.tile([P, KD, NTOK], attn_dt, name="attnT")
            # k chunks span all tokens; q chunks are per-half tiles (q-h1 all
            # six live through the overlap window, q-h0 transient).
            kT = qk_pool.tile([P, KD, NTOK], ACT, name="kT")
            # DK+4 columns: col 64 = ones (softmax denominator rides the PV
            # matmul), cols 65-67 zero pad (dual-fp8 Ldweights wants 32-bit
            # aligned row strides).
            v_aug = vaug_pool.tile([P, TC, H, DK + 4], attn_dt, name="v_aug")
            hT = h_pool.tile([P, KD, NTOK], attn_dt, name="hT")
            wv_sb = wv_pool.tile([P, KD, KD, P], wq_dt, name="wv_sb")
            x2T = x2_pool.tile([P, KD, NTOK], F32R, name="x2T")
            # h2 / g of the two halves share one slot each: h1's writes are
            # dep-serialized after h0's last reader, which matches the
            # schedule (fc1-h0 / fc2-h0 finish before the h1 tail runs).
            h2 = [
                h2_pool.tile([P, KD, F], h2_dt, tag="h2", name=f"h2_{i}")
                for i in range(2)
            ]
            gT0 = g_pool.tile([P, MF1, F], g_dt, tag="g", name="gT0")
            gT1 = g_pool.tile([P, MF1, F], g_dt, tag="g", name="gT1")
            # bf16 staging for fc1-h0's deferred (post-window) gelu, so fp8
            # gT0 is quantized once, after the nonlinearity.  bf16 gT0 can
            # take the gelu in place (no second quantization to avoid).
            g_pre = (
                g_pool.tile([P, MF1, F], ACT, tag="gpre", bufs=1, name="g_pre")
                if FC2_FP8 else gT0
            )

            # =========== prologue: load x, LN1, q/k/v ===========
            for half in range(NHALF):
                for kk in range(KD):
                    cols = slice(half * F, (half + 1) * F)
                    nc.sync.dma_start(out=xt_sb[:, kk, cols], in_=xt[:, kk, cols])

            # LN1 h0 stats, then h1 stats (PE), normalizes on DVE overlap qk.
            st0 = ln_stats(xt_sb, 0)
            st1 = ln_stats(xt_sb, 1)
            ln_norm(xt_sb, hT, 0, *st0)

            qk_w = {}
            q_tiles = {}

            def load_qk_w(j):
                wtq = wstream.tile([P, KD, P], wq_dt, tag="w", name="wtq")
                nc.sync.dma_start(out=wtq[:], in_=wqkv[j])
                wtk = wstream.tile([P, KD, P], wq_dt, tag="w", name="wtk")
                nc.sync.dma_start(out=wtk[:], in_=wqkv[KD + j])
                qk_w[j] = (wtq, wtk)

            def qk_store(dst, ps, m):
                # PSUM -> SBUF with bias; undoes the fp8 weight pre-scale.
                if ATTN_FP8:
                    nc.vector.tensor_scalar(
                        dst, ps[:], scalar1=1.0 / WSC,
                        scalar2=bqkv_sb[:, m : m + 1], op0=MUL, op1=ADD,
                    )
                else:
                    nc.vector.tensor_scalar(
                        dst, ps[:],
                        scalar1=bqkv_sb[:, m : m + 1], scalar2=None, op0=ADD,
                    )

            def wsl(t, kk, w):
                return t[:, kk, :] if w == 1 else t[:, kk : kk + w, :]

            def emit_k_chunk(j, half, wt):
                cols = slice(half * F, (half + 1) * F)
                ps = ps_tile()
                chain(ps[:], lambda kk, w: wsl(wt, kk, w),
                      lambda kk, w: hT[:, kk, cols] if w == 1
                      else hT[:, kk : kk + w, cols],
                      KD, ATTN_FP8)
                qk_store(kT[:, j, cols], ps, KD + j)

            def emit_q_chunk(j, half, wt):
                cols = slice(half * F, (half + 1) * F)
                ps = ps_tile()
                chain(ps[:], lambda kk, w: wsl(wt, kk, w),
                      lambda kk, w: hT[:, kk, cols] if w == 1
                      else hT[:, kk : kk + w, cols],
                      KD, ATTN_FP8)
                qt = ptp.tile([P, F], ACT, tag=f"q{half}",
                              bufs=(3 if half == 0 else 6), name="qt")
                q_tiles[(j, half)] = qt
                qk_store(qt[:], ps, j)

            # v projection (token-major); chunk t uses only tokens of its half.
            nc.sync.dma_start(
                out=wv_sb[:], in_=wqkv[12:18].rearrange("m p kk o -> p kk m o")
            )
            nc.vector.tensor_copy(
                out=v_aug[:, :, :, DK : DK + 4],
                in_=ones0_sb[:, None, None, :].to_broadcast([P, TC, H, 4]),
            )

            def emit_v_chunk(t):
                trange = slice(t * P, (t + 1) * P)
                ps2 = (ps_tile(), ps_tile())
                for nn in range(2):  # 512 + 256 of the 768 v features
                    nw = 512 if nn == 0 else 256
                    chain(
                        ps2[nn][:, :nw],
                        lambda kk, w: hT[:, kk, trange] if w == 1
                        else hT[:, kk : kk + w, trange],
                        lambda kk, w, nn=nn, nw=nw:
                        wv_sb[:, kk, 4 * nn : 4 * nn + nw // P, :] if w == 1
                        else wv_sb[:, kk : kk + w, 4 * nn : 4 * nn + nw // P, :],
                        KD, ATTN_FP8,
                    )
                for nn in range(2):
                    nw = 512 if nn == 0 else 256
                    hw = nw // DK
                    # v bias is folded into the proj bias on the host
                    # (proj(O + b) = proj(O) + Wproj@b), so this is a pure
                    # copy (gpsimd cannot read PSUM, so it stays on DVE).
                    nc.vector.tensor_copy(
                        out=v_aug[:, t, nn * 8 : nn * 8 + hw, 0:DK],
                        in_=ps2[nn][:, :nw].rearrange("p (h d) -> p h d", d=DK),
                    )

            load_qk_w(0)
            load_qk_w(1)
            # PE order: k/q h0 (after norm h0), first v chunks, k h1; the
            # remaining v chunks ride as fillers inside head pair 0 so the
            # Act engine starts on exps as early as possible.
            for j in (0, 1):
                emit_k_chunk(j, 0, qk_w[j][1])
                emit_q_chunk(j, 0, qk_w[j][0])
            ln_norm(xt_sb, hT, 1, *st1)
            emit_v_chunk(0)
            emit_v_chunk(1)
            for j in (0, 1):
                emit_k_chunk(j, 1, qk_w[j][1])

            # =========== attention ===========
            def emit_head_pair(j, half, fillers=()):
                fillers = list(fillers)
                cols = slice(half * F, (half + 1) * F)
                pranges = (slice(0, DK), slice(DK, P))
                q_sb = q_tiles.pop((j, half))
                o_ps = (ps_tile(), ps_tile())
                if ATTN_FP8:
                    for kcp in range(TC // 2):
                        sps = [sp_tile(), sp_tile()]
                        for par in (0, 1):
                            kc = 2 * kcp + par
                            for hi in (0, 1):
                                pr = pranges[hi]
                                nc.tensor.matmul(
                                    sps[hi][:, par, :],
                                    kT[pr, j, kc * P : (kc + 1) * P],
                                    q_sb[pr, :],
                                    start=True, stop=True,
                                )
                        pt_pairs = [
                            ptp.tile([P, 2, F], FP8, tag="pt8", name="ptpair")
                            for _ in (0, 1)
                        ]
                        for hi in (0, 1):
                            # one exp over both kc's scores (2 PSUM banks)
                            nc.scalar.activation(
                                pt_pairs[hi][:, :, :], sps[hi][:, :, :], EXP,
                                scale=float(DK) ** -0.5,
                            )
                            nc.tensor.matmul(
                                o_ps[hi][0 : DK + 4, :],
                                v_aug[:, 2 * kcp : 2 * kcp + 2, 2 * j + hi, :],
                                pt_pairs[hi][:, :, :],
                                start=(kcp == 0), stop=(kcp == TC // 2 - 1),
                                perf_mode=DR,
                            )
                        for _ in (0, 1):
                            if fillers:
                                fillers.pop(0)()
                else:
                    for kc in range(TC):
                        pts = []
                        for hi in (0, 1):
                            pr = pranges[hi]
                            s_ps = ps_tile()
                            nc.tensor.matmul(
                                s_ps[:],
                                kT[pr, j, kc * P : (kc + 1) * P],
                                q_sb[pr, :],
                                start=True, stop=True,
                            )
                            pt = ptp.tile([P, F], ACT, tag="pt", name="pt")
                            nc.scalar.activation(
                                pt[:], s_ps[:], EXP, scale=float(DK) ** -0.5
                            )
                            pts.append(pt)
                        for hi in (0, 1):
                            nc.tensor.matmul(
                                o_ps[hi][0 : DK + 4, :],
                                v_aug[:, kc, 2 * j + hi, :],
                                pts[hi][:],
                                start=(kc == 0), stop=(kc == TC - 1),
                            )
                        if fillers:
                            fillers.pop(0)()
                while fillers:
                    fillers.pop(0)()
                for hi in (0, 1):
                    rec = stat.tile([1, F], F32, tag="st", name="rec")
                    nc.vector.reciprocal(rec[:], o_ps[hi][DK : DK + 1, :])
                    rec_b = bcast.tile([DK, F], F32, tag="bc64", bufs=2, name="rec_b")
                    nc.gpsimd.partition_broadcast(rec_b[:], rec[:])
                    nc.vector.tensor_mul(
                        attnT[pranges[hi], j, cols], o_ps[hi][0:DK, :], rec_b[:]
                    )

            wp_tiles = {}

            def load_wp(m):
                wp = wstream.tile([P, KD, P], wq_dt, tag="w", name="wp")
                nc.sync.dma_start(out=wp[:], in_=wproj[m])
                wp_tiles[m] = wp

            # ---- attn h0: head pairs with v chunks (j=0) and q/k chunk
            # production (j>=1) as fillers ----
            for j in range(KD):
                if j == 0:
                    fillers = [lambda t=t: emit_v_chunk(t) for t in range(2, TC)]
                else:
                    fillers = [lambda j=j: emit_q_chunk(j, 1, qk_w[j][0])]
                    if j == 1:
                        fillers.insert(
                            0, lambda: emit_q_chunk(0, 1, qk_w[0][0])
                        )
                    if j + 1 < KD:
                        jj = j + 1

                        def _load_and_k0(jj=jj):
                            load_qk_w(jj)
                            emit_k_chunk(jj, 0, qk_w[jj][1])

                        fillers += [
                            _load_and_k0,
                            lambda jj=jj: emit_k_chunk(jj, 1, qk_w[jj][1]),
                            lambda jj=jj: emit_q_chunk(jj, 0, qk_w[jj][0]),
                        ]
                    else:
                        fillers += [lambda m=m: load_wp(m) for m in range(KD)]
                emit_head_pair(j, 0, fillers)

            # ---- window: attn h1 with proj/LN2/fc1 of h0 interleaved ----
            def emit_proj_chunk(m, half):
                cols = slice(half * F, (half + 1) * F)
                wp = wp_tiles.pop(m)
                ps = ps_tile()
                chain(ps[:], lambda kk, w: wsl(wp, kk, w),
                      lambda kk, w: attnT[:, kk, cols] if w == 1
                      else attnT[:, kk : kk + w, cols],
                      KD, ATTN_FP8)
                if ATTN_FP8:
                    # attnT carries WSC (v scale), wproj carries WSC.
                    nc.vector.tensor_scalar(
                        x2T[:, m, cols], ps[:], scalar1=1.0 / (WSC * WSC),
                        scalar2=bproj_sb[:, m : m + 1], op0=MUL, op1=ADD,
                    )
                else:
                    nc.vector.tensor_scalar(
                        x2T[:, m, cols], ps[:],
                        scalar1=bproj_sb[:, m : m + 1], scalar2=None, op0=ADD,
                    )
                nc.gpsimd.tensor_tensor(
                    x2T[:, m, cols], x2T[:, m, cols], xt_sb[:, m, cols], ADD
                )

            def emit_fc1_chunk(m, half, deferred_gelu):
                wt = wstream.tile([P, KD, P], w1_dt, tag="w", name="wt")
                nc.sync.dma_start(out=wt[:], in_=wfc1[m])
                ps = ps_tile()
                h2h = h2[half]
                chain(ps[:], lambda kk, w: wsl(wt, kk, w),
                      lambda kk, w: wsl(h2h, kk, w),
                      KD, FC1_FP8)
                gT = gT0 if half == 0 else gT1
                if deferred_gelu:
                    # DVE bias add into bf16 staging; gelu applied
                    # post-window (keeps Act on Exp in the overlap window).
                    if FC1_FP8:
                        nc.vector.tensor_scalar(
                            g_pre[:, m, :], ps[:], scalar1=1.0 / WSC,
                            scalar2=bfc1_sb[:, m : m + 1], op0=MUL, op1=ADD,
                        )
                    else:
                        nc.vector.tensor_scalar(
                            g_pre[:, m, :], ps[:],
                            scalar1=bfc1_sb[:, m : m + 1], scalar2=None, op0=ADD,
                        )
                else:
                    nc.scalar.activation(
                        gT[:, m, :], ps[:], GELU,
                        bias=bfc1_sb[:, m : m + 1],
                        scale=(1.0 / WSC if FC1_FP8 else 1.0),
                    )

            ln2_st = {}

            def ln2_stats_h0():
                ln2_st[0] = ln_stats(x2T, 0)

            def ln2_norm_h0():
                ln_norm(x2T, h2[0], 0, *ln2_st[0], dst_local=True)

            for j in range(KD):
                if j == 0:
                    fillers = [lambda m=m: emit_proj_chunk(m, 0) for m in range(KD)]
                elif j == 1:
                    fillers = [ln2_stats_h0, ln2_norm_h0]
                elif j < 5:
                    fillers = [
                        lambda m=m: emit_fc1_chunk(m, 0, True)
                        for m in range(6 * (j - 2), 6 * (j - 2) + 6)
                    ]
                else:
                    fillers = []
                emit_head_pair(j, 1, fillers)

            # ---- post-window: gelu h0, proj h1, LN2 h1, fc2 h0, MLP h1 ----
            # zdep: a zero bias tile data-dependent on the last attention
            # output, pinning the deferred gelus after the exp window so the
            # scheduler can't interleave them (Exp/Gelu table thrash).
            zdep = stat.tile([P, 1], F32, tag="zdep", bufs=1, name="zdep")
            nc.vector.tensor_scalar_mul(zdep[:], attnT[:, KD - 1, NTOK - 1 : NTOK], 0.0)
            gelu_pairs = list(range(9))  # 18 deferred chunks -> 9 pairs
            for m in range(KD):
                for mm in gelu_pairs[: 2 if m < 3 else 1]:
                    # paired gelus: one Act instruction per two chunks.
                    nc.scalar.activation(
                        gT0[:, 2 * mm : 2 * mm + 2, :],
                        g_pre[:, 2 * mm : 2 * mm + 2, :], GELU,
                        bias=zdep[:, 0:1], scale=1.0,
                    )
                    gelu_pairs.remove(mm)
                load_wp(m)
                emit_proj_chunk(m, 1)
            st2 = ln_stats(x2T, 1)
            ln_norm(x2T, h2[1], 1, *st2, dst_local=True)
            # the last six fc1-h0 chunks fill the PE dip under the gelu
            # shadow, with direct (non-deferred) gelu.
            for m in range(18, MF1):
                emit_fc1_chunk(m, 0, False)

            def emit_fc2_chunk(m, half):
                cols = slice(half * F, (half + 1) * F)
                gT = gT0 if half == 0 else gT1
                w2 = w2stream.tile([P, MF1, P], w2_dt, tag="w2", name="w2")
                nc.sync.dma_start(out=w2[:], in_=wfc2[m])
                ps = ps_tile()
                chain(ps[:], lambda kk, w: wsl(w2, kk, w),
                      lambda kk, w: wsl(gT, kk, w),
                      MF1, FC2_FP8)
                yo = outp.tile([P, F], F32, tag="yo", name="yo")
                if FC2_FP8:
                    nc.vector.tensor_scalar(
                        yo[:], ps[:], scalar1=1.0 / WSC,
                        scalar2=bfc2_sb[:, m : m + 1], op0=MUL, op1=ADD,
                    )
                else:
                    nc.vector.tensor_scalar(
                        yo[:], ps[:],
                        scalar1=bfc2_sb[:, m : m + 1], scalar2=None, op0=ADD,
                    )
                nc.vector.tensor_add(yo[:], yo[:], x2T[:, m, cols])
                nc.sync.dma_start(out=yt[:, m, cols], in_=yo[:])

            for m in range(KD):
                emit_fc2_chunk(m, 0)
            for m in range(MF1):
                emit_fc1_chunk(m, 1, False)
            for m in range(KD):
                emit_fc2_chunk(m, 1)

            g_pool.release()
            h2_pool.release()
            x2_pool.release()
            wv_pool.release()
            h_pool.release()
            vaug_pool.release()
            qk_pool.release()
            attn_pool.release()
            xt_pool.release()

        w2stream.release()
        wstream.release()
        ptp.release()
        outp.release()
        tmpp.release()
        sqp.release()
        bcast.release()
        stat.release()
        const.release()
        psum.release()

    nc.compile()
    return nc


def _retile_w(w_t, mtiles):
    """[out, in] torch-convention weight -> [mtiles, P, in//P, P] chunk layout.

    chunk[m, p, kk, o] = w_t[m*P + o, kk*P + p]
    """
    out_dim, in_dim = w_t.shape
    a = w_t.reshape(mtiles, P, in_dim // P, P).transpose(0, 3, 2, 1)
    return np.ascontiguousarray(a)


def _vec_tile(v):
    """[n] -> [P, n//P] with t[p, m] = v[m*P + p]."""
    return np.ascontiguousarray(v.reshape(-1, P).T)


_NC_CACHE = {}


def _get_nc():
    if "nc" not in _NC_CACHE:
        _NC_CACHE["nc"] = build_program()
    return _NC_CACHE["nc"]


def prep_inputs(x, ln1_w, ln1_b, qkv_w, qkv_b, proj_w, proj_b,
                ln2_w, ln2_b, fc1_w, fc1_b, fc2_w, fc2_b):
    import ml_dtypes

    wdt_np = np.dtype(ml_dtypes.bfloat16)
    fp8_np = np.dtype(mybir.dt.np(FP8))
    wq_np = fp8_np if ATTN_FP8 else wdt_np
    w1_np = fp8_np if FC1_FP8 else wdt_np
    w2_np = fp8_np if FC2_FP8 else wdt_np
    wq_sc = WSC if ATTN_FP8 else 1.0
    w1_sc = WSC if FC1_FP8 else 1.0
    w2_sc = WSC if FC2_FP8 else 1.0
    f32 = lambda a: np.asarray(a, dtype=np.float32)
    x = f32(x)

    # Fold LN gains/biases into the consuming weights, and the v bias into
    # the proj bias: attn_out(v + b) = attn_out(v) + b, so
    # proj(attn + b) + proj_b = proj(attn) + (proj_b + proj_w @ b).
    qkv_w_f = f32(qkv_w) * f32(ln1_w)[None, :]
    qkv_b_f = f32(qkv_b) + f32(qkv_w) @ f32(ln1_b)
    fc1_w_f = f32(fc1_w) * f32(ln2_w)[None, :]
    fc1_b_f = f32(fc1_b) + f32(fc1_w) @ f32(ln2_b)
    proj_b_f = f32(proj_b) + f32(proj_w) @ qkv_b_f[1536:]

    shared = {
        "onesr": np.ones((P, 1), dtype=np.float32),
        "wqkv": _retile_w(qkv_w_f * wq_sc, 18).astype(wq_np),
        "bqkv": _vec_tile(qkv_b_f),
        "wproj": _retile_w(f32(proj_w) * wq_sc, KD).astype(wq_np),
        "bproj": _vec_tile(proj_b_f),
        "wfc1": _retile_w(fc1_w_f * w1_sc, MF1).astype(w1_np),
        "bfc1": _vec_tile(fc1_b_f),
        "wfc2": _retile_w(f32(fc2_w) * w2_sc, KD).astype(w2_np),
        "bfc2": _vec_tile(f32(fc2_b)),
    }
    in_maps = []
    for b in range(N_CORES):
        m = dict(shared)
        # xt[p, s, n] = x[b, n, s*P + p]
        m["xt"] = np.ascontiguousarray(x[b].reshape(NTOK, KD, P).transpose(2, 1, 0))
        in_maps.append(m)
    return in_maps


def kernel(**inputs):
    nc = _get_nc()
    in_maps = prep_inputs(**inputs)
    res = run_bass_kernel_spmd(nc, in_maps, list(range(N_CORES)))
    outs = []
    for b in range(N_CORES):
        ytile = res.results[b]["yt"]  # [P, KD, NTOK]
        outs.append(ytile.transpose(2, 1, 0).reshape(NTOK, D))
    return np.stack(outs).astype(np.float32)
